# revision 1
# baseline (speedup 1.0000x reference)
"""Trainium2 Bass kernel for nn_CholeskyConstraintLayer.

Maps x:(B,16) f32 -> rho:(B,4,4,2) f32 where rho = L L^dagger / (trace + eps),
L lower-triangular complex 4x4 built from x (softplus diagonal, raw re/im
off-diagonals).

Per-sample math (y = x with softplus applied at flat positions 0,3,8,15):
  row0 = y[0:1], row1 = y[1:4], row2 = y[4:9], row3 = y[9:16]   (interleaved re,im; diag last)
  rho_ij_re (i>=j) = dot(row_i[0:2j+1], row_j[0:2j+1])
  rho_ij_im (i>j)  = dot(zrow_i[0:2j+1], row_j[0:2j+1])  with z = pair-swapped,
                     odd-negated y (z[2k]=y[2k+1], z[2k+1]=-y[2k] inside each row)
  diag: rho_ii = sum of squares of row_i;  trace = sum of all 16 squares.

Work split per tile (samples on partitions x free dim, 16 values contiguous per
sample): ACT does softplus/squares/z-build/upper-triangle fills; DVE does the
dot-product multiplies+adds+segment reduces+reciprocal; POOL (gpsimd) does the
j=0 column products, the in-place normalize and the zero diagonal-imag slots.
DMA via HWDGE (nc.sync) so it never contends with compute.

Data parallel over 8 NeuronCores: batch padded to 8*128*sum(F_LIST) samples,
each core gets one contiguous shard.
"""

import numpy as np

P = 128
EPS = 1e-8
N_CORES = 8
BATCH = 1_000_000
# Tapered per-tile free sizes (samples per partition per tile). Small head
# tiles fill the pipeline quickly; sum * P * N_CORES >= BATCH.
F_LIST = [60, 130, 150, 150, 150, 150, 97, 90]  # sum = 977
S_CORE = P * sum(F_LIST)  # 125056
S_PAD = S_CORE * N_CORES  # 1000448

_NC_CACHE = {}


def _emit(tc, x_ap, out_ap, f_list):
    import concourse.bass as bass
    import concourse.mybir as mybir
    from contextlib import ExitStack

    nc = tc.nc
    f32 = mybir.dt.float32
    A = mybir.AluOpType
    ACT = mybir.ActivationFunctionType

    def block_ap(view3, start, bstride, nblocks, inner, bcast=False):
        """(128,F,inner) slice at col `start` -> (128,F,nblocks,inner) blocks."""
        a = view3[:, :, start:start + inner]
        dims = [list(d) for d in a.ap]
        step = 0 if bcast else bstride
        new = dims[:2] + [[step, nblocks]] + [dims[2]]
        return bass.AP(tensor=a.tensor, offset=a.offset, ap=new)

    with ExitStack() as ctx:
        tp = lambda name, bufs: ctx.enter_context(tc.tile_pool(name=name, bufs=bufs))
        ypool = tp("y", 3)
        sqpool = tp("sq", 3)
        zpool = tp("z", 3)
        prpool = tp("pr", 3)
        mpool = tp("misc", 3)
        opool = tp("out", 3)

        s0 = 0
        for ti, F in enumerate(f_list):
            # ---- DMA in: partition p holds samples s0 + p*F .. s0 + (p+1)*F-1
            y_t = ypool.tile([P, F * 16], f32, tag="y")
            xin = bass.AP(tensor=x_ap.tensor, offset=(s0 * 16),
                          ap=[[F * 16, P], [1, F * 16]])
            nc.sync.dma_start(y_t[:, :], xin)

            yf = y_t[:, :]
            y = yf.rearrange("p (f e) -> p f e", e=16)

            # ---- ACT: softplus = Ln(Exp(x) + 1) on diagonal positions, in
            # place (sq cols used as scratch for the exp; Square later
            # overwrites all of sq from the softplus'd y).
            sq_t = sqpool.tile([P, F * 16], f32, tag="sq")
            sq = sq_t[:, :].rearrange("p (f e) -> p f e", e=16)
            # diagonal cols pair into affine APs: (3,8) step 5, (0,15) step 15
            for sl in (slice(3, 9, 5), slice(0, 16, 15)):
                nc.scalar.activation(sq[:, :, sl], y[:, :, sl], ACT.Exp)
                nc.scalar.activation(y[:, :, sl], sq[:, :, sl], ACT.Ln, bias=1.0)

            # ---- ACT: squares of everything
            nc.scalar.activation(sq_t[:, :], yf, ACT.Square)

            # ---- ACT: z = swapped/negated pairs (cols: i20,-r20,i21, i30,-r30,i31,-r31,i32)
            z_t = zpool.tile([P, F * 8], f32, tag="z")
            z = z_t[:, :].rearrange("p (f e) -> p f e", e=8)
            nc.scalar.copy(z[:, :, 0:3:2], y[:, :, 5:8:2])     # z0=y5, z2=y7
            nc.scalar.copy(z[:, :, 3:8:2], y[:, :, 10:15:2])   # z3=y10, z5=y12, z7=y14
            nc.scalar.mul(z[:, :, 1:2], y[:, :, 4:5], -1.0)    # z1=-y4
            nc.scalar.mul(z[:, :, 4:7:2], y[:, :, 9:12:2], -1.0)  # z4=-y9, z6=-y11

            # ---- DVE: off-diagonal products (TT ISA allows max 3 AP dims,
            # so one op per 3-element block)
            pr_t = prpool.tile([P, F * 22], f32, tag="pr")
            prf = pr_t[:, :]
            prv = prf.rearrange("p (f e) -> p f e", e=22)
            row1 = y[:, :, 1:4]
            nc.vector.tensor_tensor(prv[:, :, 0:3], y[:, :, 4:7], row1, op=A.mult)
            nc.vector.tensor_tensor(prv[:, :, 3:6], y[:, :, 9:12], row1, op=A.mult)
            nc.vector.tensor_tensor(prv[:, :, 6:9], z[:, :, 0:3], row1, op=A.mult)
            nc.vector.tensor_tensor(prv[:, :, 9:12], z[:, :, 3:6], row1, op=A.mult)
            nc.vector.tensor_tensor(prv[:, :, 12:17], y[:, :, 9:14], y[:, :, 4:9], op=A.mult)
            nc.vector.tensor_tensor(prv[:, :, 17:22], z[:, :, 3:8], y[:, :, 4:9], op=A.mult)

            out_t = opool.tile([P, F * 32], f32, tag="out")
            ov = out_t[:, :].rearrange("p (f e) -> p f e", e=32)

            # ---- POOL: zero the diagonal-imag slots first (no data deps --
            # absorbs the out-slot release wait while other engines work)
            nc.gpsimd.memset(ov[:, :, 1:32:10], 0.0)

            # ---- POOL: j=0 products straight into (unnormalized) out slots
            y0b = y[:, :, 0:1].broadcast_to((P, F, 2))
            nc.gpsimd.tensor_tensor(ov[:, :, 8:10], y[:, :, 1:3], y0b, op=A.mult)
            nc.gpsimd.tensor_tensor(ov[:, :, 16:18], y[:, :, 4:6], y0b, op=A.mult)
            nc.gpsimd.tensor_tensor(ov[:, :, 24:26], y[:, :, 9:11], y0b, op=A.mult)

            # ---- DVE: adds. k3: (21re,31re,21im,31im) -> slots (18,26,19,27)
            misc_t = mpool.tile([P, F * 6], f32, tag="misc")
            mv = misc_t[:, :].rearrange("p (f e) -> p f e", e=6)
            pr4 = prv[:, :, 0:12].rearrange("p f (s e) -> p f s e", e=3)
            t3 = mv[:, :, 0:4]
            nc.vector.tensor_tensor(t3, pr4[:, :, :, 0], pr4[:, :, :, 1], op=A.add)
            # final adds: (21re,31re) -> slots (18,26); (21im,31im) -> (19,27)
            dst_re = bass.AP(tensor=ov.tensor, offset=ov.offset + 18,
                             ap=[list(ov.ap[0]), [32, F], [8, 2]])
            dst_im = bass.AP(tensor=ov.tensor, offset=ov.offset + 19,
                             ap=[list(ov.ap[0]), [32, F], [8, 2]])
            nc.vector.tensor_tensor(dst_re, t3[:, :, 0:2], pr4[:, :, 0:2, 2], op=A.add)
            nc.vector.tensor_tensor(dst_im, t3[:, :, 2:4], pr4[:, :, 2:4, 2], op=A.add)
            # k5: (32re,32im) -> slots (28,29)
            pr5 = prv[:, :, 12:22].rearrange("p f (s e) -> p f s e", e=5)
            t5 = mv[:, :, 4:6]
            nc.vector.tensor_tensor(t5, pr5[:, :, :, 0], pr5[:, :, :, 1], op=A.add)
            nc.vector.tensor_tensor(t5, t5, pr5[:, :, :, 2], op=A.add)
            nc.vector.tensor_tensor(t5, t5, pr5[:, :, :, 3], op=A.add)
            nc.vector.tensor_tensor(ov[:, :, 28:30], t5, pr5[:, :, :, 4], op=A.add)

            # ---- DVE: diagonal sums of squares -> slots 10, 20, 30
            X = mybir.AxisListType.X
            nc.vector.tensor_reduce(ov[:, :, 10:11], sq[:, :, 1:4], axis=X, op=A.add)
            nc.vector.tensor_reduce(ov[:, :, 20:21], sq[:, :, 4:9], axis=X, op=A.add)
            nc.vector.tensor_reduce(ov[:, :, 30:31], sq[:, :, 9:16], axis=X, op=A.add)

            # ---- DVE: trace + eps, reciprocal
            s1, s2, trE, rcp = (mv[:, :, 0:1], mv[:, :, 1:2], mv[:, :, 2:3], mv[:, :, 3:4])
            nc.vector.tensor_tensor(s1, sq[:, :, 0:1], ov[:, :, 10:11], op=A.add)
            nc.vector.tensor_tensor(s2, s1, ov[:, :, 20:21], op=A.add)
            nc.vector.scalar_tensor_tensor(trE, ov[:, :, 30:31], float(EPS), s2,
                                           op0=A.add, op1=A.add)
            nc.vector.reciprocal_approx_fast(rcp, trE)

            # ---- POOL: normalize lower triangle + diagonal (in place)
            nc.gpsimd.tensor_tensor(ov[:, :, 0:1], sq[:, :, 0:1], rcp, op=A.mult)
            nc.gpsimd.tensor_tensor(ov[:, :, 8:11], ov[:, :, 8:11],
                                    rcp.broadcast_to((P, F, 3)), op=A.mult)
            nc.gpsimd.tensor_tensor(ov[:, :, 16:21], ov[:, :, 16:21],
                                    rcp.broadcast_to((P, F, 5)), op=A.mult)
            nc.gpsimd.tensor_tensor(ov[:, :, 24:31], ov[:, :, 24:31],
                                    rcp.broadcast_to((P, F, 7)), op=A.mult)

            # ---- ACT: upper triangle from normalized lower (conjugate)
            nc.scalar.copy(ov[:, :, 2:7:2], ov[:, :, 8:25:8])      # re row0
            nc.scalar.copy(ov[:, :, 12:15:2], ov[:, :, 18:27:8])   # re (1,2),(1,3)
            nc.scalar.copy(ov[:, :, 22:23], ov[:, :, 28:29])       # re (2,3)
            nc.scalar.mul(ov[:, :, 3:8:2], ov[:, :, 9:26:8], -1.0)   # im row0
            nc.scalar.mul(ov[:, :, 13:16:2], ov[:, :, 19:28:8], -1.0)
            nc.scalar.mul(ov[:, :, 23:24], ov[:, :, 29:30], -1.0)

            # ---- DMA out
            odst = bass.AP(tensor=out_ap.tensor, offset=(s0 * 32),
                           ap=[[F * 32, P], [1, F * 32]])
            nc.sync.dma_start(odst, out_t[:, :])

            s0 += P * F


def _patch_act_tables():
    """Force every ACT function onto the one table set that contains all of
    Exp/Ln/Square/Copy, so the table-load pass emits a single load instead of
    ping-ponging exp_and_others <-> natural_log every tile (~2.7us per load).
    Keys keep their order so act_func_set_id indices stay valid."""
    import concourse.bacc as bacc
    from concourse.hw_specs import get_activation_tables as _orig

    if getattr(bacc, "_act_tables_patched", False):
        return

    def _patched(arch):
        t = _orig(arch)
        return {k: (v if k == "natural_log_exp_and_others" else set())
                for k, v in t.items()}

    bacc.get_activation_tables = _patched
    bacc._act_tables_patched = True


def _build_nc(f_list):
    import concourse.bacc as bacc
    import concourse.mybir as mybir
    import concourse.tile as tile

    _patch_act_tables()

    key = tuple(f_list)
    if key in _NC_CACHE:
        return _NC_CACHE[key]
    S = P * sum(f_list)
    nc = bacc.Bacc("TRN2", target_bir_lowering=False, debug=False)
    x = nc.dram_tensor("x", (S, 16), mybir.dt.float32, kind="ExternalInput")
    out = nc.dram_tensor("out", (S, 32), mybir.dt.float32, kind="ExternalOutput")
    with tile.TileContext(nc) as tc:
        _emit(tc, x.ap(), out.ap(), f_list)
    nc.compile()
    _NC_CACHE[key] = nc
    return nc


def kernel(x, _trace=False):
    from concourse.bass_utils import run_bass_kernel_spmd

    x = np.ascontiguousarray(np.asarray(x, dtype=np.float32))
    B = x.shape[0]
    assert x.shape == (B, 16) and B <= S_PAD
    xp = np.zeros((S_PAD, 16), dtype=np.float32)
    xp[:B] = x
    shards = xp.reshape(N_CORES, S_CORE, 16)
    nc = _build_nc(F_LIST)
    in_maps = [{"x": np.ascontiguousarray(shards[i])} for i in range(N_CORES)]
    res = run_bass_kernel_spmd(nc, in_maps, core_ids=list(range(N_CORES)),
                               trace=_trace)
    out = np.concatenate([r["out"].reshape(S_CORE, 32) for r in res.results], axis=0)
    result = out[:B].reshape(B, 4, 4, 2)
    if _trace:
        return result, res
    return result



# revision 2
# speedup vs baseline: 1.5149x; 1.5149x over previous
"""Trainium2 Bass kernel for nn_CholeskyConstraintLayer — fp16 16-unique-value
design, software-pipelined emission.

Maps x:(B,16) f32 -> rho:(B,4,4,2) f32 where rho = L L^dagger / (trace + eps),
L lower-triangular complex 4x4 built from x (softplus diagonal, raw re/im
off-diagonals).

The device computes, per sample, the 16 unique values of the Hermitian rho in
the slot layout
  [r10,i10, r20,i20, r30,i30, re21,re31, im21,im31, re32,im32, d0,d1,d2,d3]
(chosen so every on-device writer gets a packed access pattern).  The host
casts fp16 -> f32 and mirrors the conjugate-symmetric full (B,4,4,2) tensor
(upper triangle = conj of lower, diagonal imag = 0).

All device IO is fp16 (input cast host-side): 32B in + 32B out per sample.
Work split: ACT does softplus (exp+ln), squares and the pair-swap copies; DVE
does the products, level-1 dot adds, diag-sum trees, trace/reciprocal and the
packed in-place normalize; POOL does the j0-column products, level-2/3 dot
adds and the s1 tree.

Emission is software-pipelined: the serial per-tile tail (trace -> rcp ->
normalize -> DMA out, all on DVE after two POOL partials) of tile t is
emitted between the ACT phase of tile t+1 and the products phase of tile t+1,
so no in-order engine queue ever stalls the next tile's work behind the tail
chain.

Per-sample math (y = x with softplus at 0,3,8,15; zsw = [y5,y4,y7,
y10,y9,y12,y11,y14]):
  re21,im21 from products y[4:7]*y[1:4] and zsw[0:3]*y[1:4] (im = p0-p1+p2)
  re31,im31 from y[9:12]*y[1:4] and zsw[3:6]*y[1:4]
  re32,im32 from y[9:14]*y[4:9] and zsw[3:8]*y[4:9] (im = p0-p1+p2-p3+p4)
  (i,0) column: (y1,y2)*y0, (y4,y5)*y0, (y9,y10)*y0
  d_i = sum of squares of row i;  trace = sum of all 16 squares.
"""

import numpy as np

P = 128
EPS = 1e-8
N_CORES = 8
BATCH = 1_000_000
F_LIST = [80, 220, 220, 220, 177, 60]  # sum = 977
S_CORE = P * sum(F_LIST)  # 125056
S_PAD = S_CORE * N_CORES  # 1000448

_NC_CACHE = {}

# Host-side conjugate mirror indices into the device slot layout.
_RE_IDX = np.array([[12, 0, 2, 4], [0, 13, 6, 7], [2, 6, 14, 10], [4, 7, 10, 15]])
_IM_IDX = np.array([[0, 1, 3, 5], [1, 0, 8, 9], [3, 8, 0, 11], [5, 9, 11, 0]])
_IM_SGN = np.array([[0., -1., -1., -1.], [1., 0., -1., -1.],
                    [1., 1., 0., -1.], [1., 1., 1., 0.]], dtype=np.float32)


def _emit(tc, x_ap, out_ap, f_list):
    import concourse.bass as bass
    import concourse.mybir as mybir
    from contextlib import ExitStack

    nc = tc.nc
    f16 = mybir.dt.float16
    f32 = mybir.dt.float32
    A = mybir.AluOpType
    ACT = mybir.ActivationFunctionType

    def ap3(view, offset, dims):
        """Custom AP rooted at a (P, F, e) view: dims = [stride, n] pairs in
        element units appended after the partition dim."""
        return bass.AP(tensor=view.tensor, offset=view.offset + offset,
                       ap=[list(view.ap[0])] + [list(d) for d in dims])

    starts = []
    s0 = 0
    for F in f_list:
        starts.append(s0)
        s0 += P * F

    with ExitStack() as ctx:
        tp = lambda name, bufs: ctx.enter_context(tc.tile_pool(name=name, bufs=bufs))
        ypool = tp("y", 3)
        sqpool = tp("sq", 3)
        zpool = tp("zsw", 3)
        prpool = tp("pr", 3)
        hpool = tp("h", 3)
        mpool = tp("misc", 3)
        opool = tp("out", 3)

        live = {}

        def emit_phase_a(ti):
            """DMA-in + all ACT work (softplus, squares, zsw)."""
            F = f_list[ti]
            s0 = starts[ti]
            y_t = ypool.tile([P, F * 16], f16, tag="y")
            xin = bass.AP(tensor=x_ap.tensor, offset=(s0 * 16),
                          ap=[[F * 16, P], [1, F * 16]])
            nc.sync.dma_start(y_t[:, :], xin)

            y = y_t[:, :].rearrange("p (f e) -> p f e", e=16)

            # softplus = Ln(Exp(x) + 1) on diag slots (sq as scratch)
            sq_t = sqpool.tile([P, F * 16], f16, tag="sq")
            sq = sq_t[:, :].rearrange("p (f e) -> p f e", e=16)
            for sl in (slice(3, 9, 5), slice(0, 16, 15)):
                nc.scalar.activation(sq[:, :, sl], y[:, :, sl], ACT.Exp)
                nc.scalar.activation(y[:, :, sl], sq[:, :, sl], ACT.Ln, bias=1.0)

            # squares: slots 1..15 into sq, sq0 straight into d0@12
            nc.scalar.activation(sq[:, :, 1:16], y[:, :, 1:16], ACT.Square)

            o_t = opool.tile([P, F * 16], f16, tag="o")
            o = o_t[:, :].rearrange("p (f e) -> p f e", e=16)
            nc.scalar.activation(o[:, :, 12:13], y[:, :, 0:1], ACT.Square)

            # zsw = [y5,y4,y7, y10,y9,y12,y11,y14] (pure copies; negations
            # fold into the subtract adds)
            z_t = zpool.tile([P, F * 8], f16, tag="z")
            z = z_t[:, :].rearrange("p (f e) -> p f e", e=8)
            nc.scalar.copy(ap3(z, 0, [[8, F], [1, 2]]),
                           ap3(y, 5, [[16, F], [-1, 2]]))          # y5,y4
            nc.scalar.copy(z[:, :, 2:3], y[:, :, 7:8])              # y7
            nc.scalar.copy(ap3(z, 3, [[8, F], [2, 2], [1, 2]]),
                           ap3(y, 10, [[16, F], [2, 2], [-1, 2]]))  # y10,y9,y12,y11
            nc.scalar.copy(z[:, :, 7:8], y[:, :, 14:15])            # y14

            live[ti] = dict(y=y, sq=sq, z=z, o=o, o_t=o_t, F=F, s0=s0)

        def emit_phase_b(ti):
            """Products, tree adds, diag sums (DVE + POOL)."""
            d = live[ti]
            y, sq, z, o, F = d["y"], d["sq"], d["z"], d["o"], d["F"]

            # POOL: j0 column, merged pairs: (r10,i10,r20,i20) then (r30,i30)
            y0b = y[:, :, 0:1].broadcast_to((P, F, 2))
            nc.gpsimd.tensor_tensor(o[:, :, 0:2], y[:, :, 1:3], y0b, op=A.mult)
            nc.gpsimd.tensor_tensor(o[:, :, 2:4], y[:, :, 4:6], y0b, op=A.mult)
            nc.gpsimd.tensor_tensor(o[:, :, 4:6], y[:, :, 9:11], y0b, op=A.mult)

            # DVE products into pr:
            # re21@0(3) im21@3(3) re31@6(3) im31@9(3) re32@12(5) im32@17(5)
            pr_t = prpool.tile([P, F * 22], f16, tag="pr")
            pr = pr_t[:, :].rearrange("p (f e) -> p f e", e=22)
            row1 = y[:, :, 1:4]
            nc.vector.tensor_tensor(pr[:, :, 0:3], y[:, :, 4:7], row1, op=A.mult)
            nc.vector.tensor_tensor(pr[:, :, 3:6], z[:, :, 0:3], row1, op=A.mult)
            nc.vector.tensor_tensor(pr[:, :, 6:9], y[:, :, 9:12], row1, op=A.mult)
            nc.vector.tensor_tensor(pr[:, :, 9:12], z[:, :, 3:6], row1, op=A.mult)
            row2 = y[:, :, 4:9]
            nc.vector.tensor_tensor(pr[:, :, 12:17], y[:, :, 9:14], row2, op=A.mult)
            nc.vector.tensor_tensor(pr[:, :, 17:22], z[:, :, 3:8], row2, op=A.mult)

            # tree adds.  h layout: k3h@0(4) k5h@4(4) k5l2@8(2)
            h_t = hpool.tile([P, F * 10], f16, tag="h")
            h = h_t[:, :].rearrange("p (f e) -> p f e", e=10)
            # k3 L1 (DVE): h[d] = p0 + p2 for dots (re21, im21, re31, im31)
            nc.vector.tensor_tensor(h[:, :, 0:4],
                                    ap3(pr, 0, [[22, F], [3, 4]]),
                                    ap3(pr, 2, [[22, F], [3, 4]]), op=A.add)
            # k5 L1 (DVE): (p0,p1)+(p2,p3) for re32, im32
            nc.vector.tensor_tensor(h[:, :, 4:8],
                                    ap3(pr, 12, [[22, F], [5, 2], [1, 2]]),
                                    ap3(pr, 14, [[22, F], [5, 2], [1, 2]]), op=A.add)
            # k3 L2 (POOL): (re21,re31)@6 = h{0,2}+pr{1,7}; (im21,im31)@8 = h{1,3}-pr{4,10}
            nc.gpsimd.tensor_tensor(o[:, :, 6:8],
                                    ap3(h, 0, [[10, F], [2, 2]]),
                                    ap3(pr, 1, [[22, F], [6, 2]]), op=A.add)
            nc.gpsimd.tensor_tensor(o[:, :, 8:10],
                                    ap3(h, 1, [[10, F], [2, 2]]),
                                    ap3(pr, 4, [[22, F], [6, 2]]), op=A.subtract)
            # k5 L2 (POOL): h8 = h4+h5 (re), h9 = h6-h7 (im)
            nc.gpsimd.tensor_tensor(h[:, :, 8:9], h[:, :, 4:5], h[:, :, 5:6], op=A.add)
            nc.gpsimd.tensor_tensor(h[:, :, 9:10], h[:, :, 6:7], h[:, :, 7:8], op=A.subtract)
            # k5 L3 (POOL): (re32,im32)@10 = h[8:10] + pr{16,21}
            nc.gpsimd.tensor_tensor(o[:, :, 10:12], h[:, :, 8:10],
                                    ap3(pr, 16, [[22, F], [5, 2]]), op=A.add)

            # diag sums: s1 tree on POOL, s2/s3 trees on DVE
            # m layout: u@0, g2@(1,2), v@3, g3@(4,5,6), w@7, tp@(8,9)
            m_t = mpool.tile([P, F * 10], f16, tag="m")
            mv = m_t[:, :].rearrange("p (f e) -> p f e", e=10)
            u = mv[:, :, 0:1]
            nc.gpsimd.tensor_tensor(u, sq[:, :, 1:2], sq[:, :, 2:3], op=A.add)
            nc.gpsimd.tensor_tensor(o[:, :, 13:14], u, sq[:, :, 3:4], op=A.add)
            # s2 = sq4+..+sq8, s3 = sq9+..+sq15
            nc.vector.tensor_tensor(mv[:, :, 1:3], sq[:, :, 4:6], sq[:, :, 6:8], op=A.add)
            nc.vector.tensor_tensor(mv[:, :, 4:7], sq[:, :, 9:12], sq[:, :, 12:15], op=A.add)
            # v = g2a+g2b ; w = g3a+g3b   (one op via stride-3 pairs)
            nc.vector.tensor_tensor(ap3(mv, 3, [[10, F], [4, 2]]),
                                    ap3(mv, 1, [[10, F], [3, 2]]),
                                    ap3(mv, 2, [[10, F], [3, 2]]), op=A.add)
            # sq15 += g3c  (so the packed L3 reads (sq8, sq15))
            nc.vector.tensor_tensor(sq[:, :, 15:16], mv[:, :, 6:7], sq[:, :, 15:16], op=A.add)
            # (d2,d3)@14 = (v, w) + (sq8, sq15)
            nc.vector.tensor_tensor(o[:, :, 14:16],
                                    ap3(mv, 3, [[10, F], [4, 2]]),
                                    ap3(sq, 8, [[16, F], [7, 2]]), op=A.add)
            d["mv"] = mv

        def emit_tail(ti):
            """Trace -> reciprocal -> normalize -> DMA out, all on DVE (single
            stream, no cross-engine queue-head stalls)."""
            d = live.pop(ti)
            o, o_t, F, s0, mv = d["o"], d["o_t"], d["F"], d["s0"], d["mv"]
            # tp = (d0+d2, d1+d3) over the packed diag block
            tp_ = mv[:, :, 8:10]
            nc.vector.tensor_tensor(tp_, o[:, :, 12:14], o[:, :, 14:16], op=A.add)
            mf_t = mpool.tile([P, F * 2], f32, tag="mf")
            mf = mf_t[:, :].rearrange("p (f e) -> p f e", e=2)
            trE, rcp = mf[:, :, 0:1], mf[:, :, 1:2]
            nc.vector.scalar_tensor_tensor(trE, mv[:, :, 8:9], float(EPS),
                                           mv[:, :, 9:10], op0=A.add, op1=A.add)
            nc.vector.reciprocal_approx_fast(rcp, trE)

            # rp: fp16 duplicated pair of rcp for the packed normalize bcast
            rp_t = mpool.tile([P, F * 2], f16, tag="rp")
            rp = rp_t[:, :].rearrange("p (f e) -> p f e", e=2)
            nc.vector.tensor_copy(rp[:, :, 0:2], rcp.broadcast_to((P, F, 2)))

            # normalize all 16 slots in place
            rpb = bass.AP(tensor=rp.tensor, offset=rp.offset,
                          ap=[list(rp.ap[0]), [2, F], [0, 8], [1, 2]])
            ov16 = bass.AP(tensor=o.tensor, offset=o.offset,
                           ap=[list(o.ap[0]), [16, F], [2, 8], [1, 2]])
            nc.vector.tensor_tensor(ov16, ov16, rpb, op=A.mult)

            # DMA out, triggered from the ACT queue (keeps SP free-running for
            # the in-DMAs; ACT reaches this after the next tile's phase A, by
            # which time the normalize is done)
            odst = bass.AP(tensor=out_ap.tensor, offset=(s0 * 16),
                           ap=[[F * 16, P], [1, F * 16]])
            nc.scalar.dma_start(odst, o_t[:, :])

        T = len(f_list)
        for t in range(T + 1):
            if t < T:
                emit_phase_a(t)
            if t >= 1:
                emit_tail(t - 1)
            if t < T:
                emit_phase_b(t)


def _patch_act_tables():
    """Force every ACT function onto the one table set that contains all of
    Exp/Ln/Square/Copy, so the table-load pass emits a single load instead of
    ping-ponging exp_and_others <-> natural_log every tile (~2.7us per load).
    Keys keep their order so act_func_set_id indices stay valid."""
    import concourse.bacc as bacc
    from concourse.hw_specs import get_activation_tables as _orig

    if getattr(bacc, "_act_tables_patched", False):
        return

    def _patched(arch):
        t = _orig(arch)
        return {k: (v if k == "natural_log_exp_and_others" else set())
                for k, v in t.items()}

    bacc.get_activation_tables = _patched
    bacc._act_tables_patched = True


def _build_nc(f_list):
    import concourse.bacc as bacc
    import concourse.mybir as mybir
    import concourse.tile as tile

    _patch_act_tables()

    key = tuple(f_list)
    if key in _NC_CACHE:
        return _NC_CACHE[key]
    S = P * sum(f_list)
    nc = bacc.Bacc("TRN2", target_bir_lowering=False, debug=False)
    x = nc.dram_tensor("x", (S, 16), mybir.dt.float16, kind="ExternalInput")
    out = nc.dram_tensor("out", (S, 16), mybir.dt.float16, kind="ExternalOutput")
    with nc.allow_low_precision(reason="fp16 kernel; 2e-2 rel-err budget"):
        with tile.TileContext(nc) as tc:
            _emit(tc, x.ap(), out.ap(), f_list)
    nc.compile()
    _NC_CACHE[key] = nc
    return nc


def kernel(x, _trace=False):
    from concourse.bass_utils import run_bass_kernel_spmd

    x = np.asarray(x)
    B = x.shape[0]
    assert x.shape == (B, 16) and B <= S_PAD
    xp = np.zeros((S_PAD, 16), dtype=np.float16)
    xp[:B] = x.astype(np.float16)
    shards = xp.reshape(N_CORES, S_CORE, 16)
    nc = _build_nc(F_LIST)
    in_maps = [{"x": np.ascontiguousarray(shards[i])} for i in range(N_CORES)]
    res = run_bass_kernel_spmd(nc, in_maps, core_ids=list(range(N_CORES)),
                               trace=_trace)
    o16 = np.concatenate([r["out"].reshape(S_CORE, 16) for r in res.results],
                         axis=0)[:B].astype(np.float32)
    full = np.empty((B, 4, 4, 2), dtype=np.float32)
    full[..., 0] = o16[:, _RE_IDX]
    full[..., 1] = o16[:, _IM_IDX] * _IM_SGN
    if _trace:
        return full, res
    return full


# revision 3
# speedup vs baseline: 1.5174x; 1.0017x over previous
"""Trainium2 Bass kernel for nn_CholeskyConstraintLayer — fp16 16-unique-value
design, software-pipelined emission.

Maps x:(B,16) f32 -> rho:(B,4,4,2) f32 where rho = L L^dagger / (trace + eps),
L lower-triangular complex 4x4 built from x (softplus diagonal, raw re/im
off-diagonals).

The device computes, per sample, the 16 unique values of the Hermitian rho in
the slot layout
  [r10,i10, r20,i20, r30,i30, re21,re31, im21,im31, re32,im32, d0,d1,d2,d3]
(chosen so every on-device writer gets a packed access pattern).  The host
casts fp16 -> f32 and mirrors the conjugate-symmetric full (B,4,4,2) tensor
(upper triangle = conj of lower, diagonal imag = 0).

All device IO is fp16 (input cast host-side): 32B in + 32B out per sample.
Work split: ACT does softplus (exp+ln), squares and the pair-swap copies; DVE
does the products, level-1 dot adds, diag-sum trees, trace/reciprocal and the
packed in-place normalize; POOL does the j0-column products, level-2/3 dot
adds and the s1 tree.

Emission is software-pipelined: the serial per-tile tail (trace -> rcp ->
normalize -> DMA out, all on DVE after two POOL partials) of tile t is
emitted between the ACT phase of tile t+1 and the products phase of tile t+1,
so no in-order engine queue ever stalls the next tile's work behind the tail
chain.

Per-sample math (y = x with softplus at 0,3,8,15; zsw = [y5,y4,y7,
y10,y9,y12,y11,y14]):
  re21,im21 from products y[4:7]*y[1:4] and zsw[0:3]*y[1:4] (im = p0-p1+p2)
  re31,im31 from y[9:12]*y[1:4] and zsw[3:6]*y[1:4]
  re32,im32 from y[9:14]*y[4:9] and zsw[3:8]*y[4:9] (im = p0-p1+p2-p3+p4)
  (i,0) column: (y1,y2)*y0, (y4,y5)*y0, (y9,y10)*y0
  d_i = sum of squares of row i;  trace = sum of all 16 squares.
"""

import numpy as np

P = 128
EPS = 1e-8
N_CORES = 8
BATCH = 1_000_000
F_LIST = [60, 210, 230, 230, 187, 60]  # sum = 977
S_CORE = P * sum(F_LIST)  # 125056
S_PAD = S_CORE * N_CORES  # 1000448

_NC_CACHE = {}

# Host-side conjugate mirror indices into the device slot layout.
_RE_IDX = np.array([[12, 0, 2, 4], [0, 13, 6, 7], [2, 6, 14, 10], [4, 7, 10, 15]])
_IM_IDX = np.array([[0, 1, 3, 5], [1, 0, 8, 9], [3, 8, 0, 11], [5, 9, 11, 0]])
_IM_SGN = np.array([[0., -1., -1., -1.], [1., 0., -1., -1.],
                    [1., 1., 0., -1.], [1., 1., 1., 0.]], dtype=np.float32)


def _emit(tc, x_ap, out_ap, f_list):
    import concourse.bass as bass
    import concourse.mybir as mybir
    from contextlib import ExitStack

    nc = tc.nc
    f16 = mybir.dt.float16
    f32 = mybir.dt.float32
    A = mybir.AluOpType
    ACT = mybir.ActivationFunctionType

    def ap3(view, offset, dims):
        """Custom AP rooted at a (P, F, e) view: dims = [stride, n] pairs in
        element units appended after the partition dim."""
        return bass.AP(tensor=view.tensor, offset=view.offset + offset,
                       ap=[list(view.ap[0])] + [list(d) for d in dims])

    starts = []
    s0 = 0
    for F in f_list:
        starts.append(s0)
        s0 += P * F

    with ExitStack() as ctx:
        tp = lambda name, bufs: ctx.enter_context(tc.tile_pool(name=name, bufs=bufs))
        ypool = tp("y", 3)
        sqpool = tp("sq", 3)
        zpool = tp("zsw", 3)
        prpool = tp("pr", 3)
        hpool = tp("h", 3)
        mpool = tp("misc", 3)
        opool = tp("out", 3)

        live = {}

        def emit_phase_a(ti):
            """DMA-in + all ACT work (softplus, squares, zsw)."""
            F = f_list[ti]
            s0 = starts[ti]
            y_t = ypool.tile([P, F * 16], f16, tag="y")
            xin = bass.AP(tensor=x_ap.tensor, offset=(s0 * 16),
                          ap=[[F * 16, P], [1, F * 16]])
            nc.sync.dma_start(y_t[:, :], xin)

            y = y_t[:, :].rearrange("p (f e) -> p f e", e=16)

            # softplus = Ln(Exp(x) + 1) on diag slots (sq as scratch)
            sq_t = sqpool.tile([P, F * 16], f16, tag="sq")
            sq = sq_t[:, :].rearrange("p (f e) -> p f e", e=16)
            for sl in (slice(3, 9, 5), slice(0, 16, 15)):
                nc.scalar.activation(sq[:, :, sl], y[:, :, sl], ACT.Exp)
                nc.scalar.activation(y[:, :, sl], sq[:, :, sl], ACT.Ln, bias=1.0)

            # squares: slots 1..15 into sq, sq0 straight into d0@12
            nc.scalar.activation(sq[:, :, 1:16], y[:, :, 1:16], ACT.Square)

            o_t = opool.tile([P, F * 16], f16, tag="o")
            o = o_t[:, :].rearrange("p (f e) -> p f e", e=16)
            nc.scalar.activation(o[:, :, 12:13], y[:, :, 0:1], ACT.Square)

            # zsw = [y5,y4,y7, y10,y9,y12,y11,y14] (pure copies; negations
            # fold into the subtract adds)
            z_t = zpool.tile([P, F * 8], f16, tag="z")
            z = z_t[:, :].rearrange("p (f e) -> p f e", e=8)
            nc.scalar.copy(ap3(z, 0, [[8, F], [1, 2]]),
                           ap3(y, 5, [[16, F], [-1, 2]]))          # y5,y4
            nc.scalar.copy(z[:, :, 2:3], y[:, :, 7:8])              # y7
            nc.scalar.copy(ap3(z, 3, [[8, F], [2, 2], [1, 2]]),
                           ap3(y, 10, [[16, F], [2, 2], [-1, 2]]))  # y10,y9,y12,y11
            nc.scalar.copy(z[:, :, 7:8], y[:, :, 14:15])            # y14

            live[ti] = dict(y=y, sq=sq, z=z, o=o, o_t=o_t, F=F, s0=s0)

        def emit_phase_b(ti):
            """Products, tree adds, diag sums (DVE + POOL)."""
            d = live[ti]
            y, sq, z, o, F = d["y"], d["sq"], d["z"], d["o"], d["F"]

            # POOL: j0 column, merged pairs: (r10,i10,r20,i20) then (r30,i30)
            y0b = y[:, :, 0:1].broadcast_to((P, F, 2))
            nc.gpsimd.tensor_tensor(o[:, :, 0:2], y[:, :, 1:3], y0b, op=A.mult)
            nc.gpsimd.tensor_tensor(o[:, :, 2:4], y[:, :, 4:6], y0b, op=A.mult)
            nc.gpsimd.tensor_tensor(o[:, :, 4:6], y[:, :, 9:11], y0b, op=A.mult)

            # DVE products into pr:
            # re21@0(3) im21@3(3) re31@6(3) im31@9(3) re32@12(5) im32@17(5)
            pr_t = prpool.tile([P, F * 22], f16, tag="pr")
            pr = pr_t[:, :].rearrange("p (f e) -> p f e", e=22)
            row1 = y[:, :, 1:4]
            nc.vector.tensor_tensor(pr[:, :, 0:3], y[:, :, 4:7], row1, op=A.mult)
            nc.vector.tensor_tensor(pr[:, :, 3:6], z[:, :, 0:3], row1, op=A.mult)
            nc.vector.tensor_tensor(pr[:, :, 6:9], y[:, :, 9:12], row1, op=A.mult)
            nc.vector.tensor_tensor(pr[:, :, 9:12], z[:, :, 3:6], row1, op=A.mult)
            row2 = y[:, :, 4:9]
            nc.vector.tensor_tensor(pr[:, :, 12:17], y[:, :, 9:14], row2, op=A.mult)
            nc.vector.tensor_tensor(pr[:, :, 17:22], z[:, :, 3:8], row2, op=A.mult)

            # tree adds.  h layout: k3h@0(4) k5h@4(4) k5l2@8(2)
            h_t = hpool.tile([P, F * 10], f16, tag="h")
            h = h_t[:, :].rearrange("p (f e) -> p f e", e=10)
            # k3 L1 (DVE): h[d] = p0 + p2 for dots (re21, im21, re31, im31)
            nc.vector.tensor_tensor(h[:, :, 0:4],
                                    ap3(pr, 0, [[22, F], [3, 4]]),
                                    ap3(pr, 2, [[22, F], [3, 4]]), op=A.add)
            # k5 L1 (DVE): (p0,p1)+(p2,p3) for re32, im32
            nc.vector.tensor_tensor(h[:, :, 4:8],
                                    ap3(pr, 12, [[22, F], [5, 2], [1, 2]]),
                                    ap3(pr, 14, [[22, F], [5, 2], [1, 2]]), op=A.add)
            # k3 L2 (POOL): (re21,re31)@6 = h{0,2}+pr{1,7}; (im21,im31)@8 = h{1,3}-pr{4,10}
            nc.gpsimd.tensor_tensor(o[:, :, 6:8],
                                    ap3(h, 0, [[10, F], [2, 2]]),
                                    ap3(pr, 1, [[22, F], [6, 2]]), op=A.add)
            nc.gpsimd.tensor_tensor(o[:, :, 8:10],
                                    ap3(h, 1, [[10, F], [2, 2]]),
                                    ap3(pr, 4, [[22, F], [6, 2]]), op=A.subtract)
            # k5 L2 (POOL): h8 = h4+h5 (re), h9 = h6-h7 (im)
            nc.gpsimd.tensor_tensor(h[:, :, 8:9], h[:, :, 4:5], h[:, :, 5:6], op=A.add)
            nc.gpsimd.tensor_tensor(h[:, :, 9:10], h[:, :, 6:7], h[:, :, 7:8], op=A.subtract)
            # k5 L3 (POOL): (re32,im32)@10 = h[8:10] + pr{16,21}
            nc.gpsimd.tensor_tensor(o[:, :, 10:12], h[:, :, 8:10],
                                    ap3(pr, 16, [[22, F], [5, 2]]), op=A.add)

            # diag sums: s1 tree on POOL, s2/s3 trees on DVE
            # m layout: u@0, g2@(1,2), v@3, g3@(4,5,6), w@7, tp@(8,9)
            m_t = mpool.tile([P, F * 10], f16, tag="m")
            mv = m_t[:, :].rearrange("p (f e) -> p f e", e=10)
            u = mv[:, :, 0:1]
            nc.gpsimd.tensor_tensor(u, sq[:, :, 1:2], sq[:, :, 2:3], op=A.add)
            nc.gpsimd.tensor_tensor(o[:, :, 13:14], u, sq[:, :, 3:4], op=A.add)
            # s2 = sq4+..+sq8, s3 = sq9+..+sq15
            nc.vector.tensor_tensor(mv[:, :, 1:3], sq[:, :, 4:6], sq[:, :, 6:8], op=A.add)
            nc.vector.tensor_tensor(mv[:, :, 4:7], sq[:, :, 9:12], sq[:, :, 12:15], op=A.add)
            # v = g2a+g2b ; w = g3a+g3b   (one op via stride-3 pairs)
            nc.vector.tensor_tensor(ap3(mv, 3, [[10, F], [4, 2]]),
                                    ap3(mv, 1, [[10, F], [3, 2]]),
                                    ap3(mv, 2, [[10, F], [3, 2]]), op=A.add)
            # sq15 += g3c  (so the packed L3 reads (sq8, sq15))
            nc.vector.tensor_tensor(sq[:, :, 15:16], mv[:, :, 6:7], sq[:, :, 15:16], op=A.add)
            # (d2,d3)@14 = (v, w) + (sq8, sq15)
            nc.vector.tensor_tensor(o[:, :, 14:16],
                                    ap3(mv, 3, [[10, F], [4, 2]]),
                                    ap3(sq, 8, [[16, F], [7, 2]]), op=A.add)
            d["mv"] = mv

        def emit_tail(ti):
            """Trace -> reciprocal -> normalize -> DMA out, all on DVE (single
            stream, no cross-engine queue-head stalls)."""
            d = live.pop(ti)
            o, o_t, F, s0, mv = d["o"], d["o_t"], d["F"], d["s0"], d["mv"]
            # tp = (d0+d2, d1+d3) over the packed diag block
            tp_ = mv[:, :, 8:10]
            nc.vector.tensor_tensor(tp_, o[:, :, 12:14], o[:, :, 14:16], op=A.add)
            mf_t = mpool.tile([P, F * 2], f32, tag="mf")
            mf = mf_t[:, :].rearrange("p (f e) -> p f e", e=2)
            trE, rcp = mf[:, :, 0:1], mf[:, :, 1:2]
            nc.vector.scalar_tensor_tensor(trE, mv[:, :, 8:9], float(EPS),
                                           mv[:, :, 9:10], op0=A.add, op1=A.add)
            nc.vector.reciprocal_approx_fast(rcp, trE)

            # rp: fp16 duplicated pair of rcp for the packed normalize bcast
            rp_t = mpool.tile([P, F * 2], f16, tag="rp")
            rp = rp_t[:, :].rearrange("p (f e) -> p f e", e=2)
            nc.vector.tensor_copy(rp[:, :, 0:2], rcp.broadcast_to((P, F, 2)))

            # normalize all 16 slots in place
            rpb = bass.AP(tensor=rp.tensor, offset=rp.offset,
                          ap=[list(rp.ap[0]), [2, F], [0, 8], [1, 2]])
            ov16 = bass.AP(tensor=o.tensor, offset=o.offset,
                           ap=[list(o.ap[0]), [16, F], [2, 8], [1, 2]])
            nc.vector.tensor_tensor(ov16, ov16, rpb, op=A.mult)

            # DMA out, triggered from the ACT queue (keeps SP free-running for
            # the in-DMAs; ACT reaches this after the next tile's phase A, by
            # which time the normalize is done)
            odst = bass.AP(tensor=out_ap.tensor, offset=(s0 * 16),
                           ap=[[F * 16, P], [1, F * 16]])
            nc.scalar.dma_start(odst, o_t[:, :])

        T = len(f_list)
        for t in range(T + 1):
            if t < T:
                emit_phase_a(t)
            if t >= 1:
                emit_tail(t - 1)
            if t < T:
                emit_phase_b(t)


def _patch_act_tables():
    """Force every ACT function onto the one table set that contains all of
    Exp/Ln/Square/Copy, so the table-load pass emits a single load instead of
    ping-ponging exp_and_others <-> natural_log every tile (~2.7us per load).
    Keys keep their order so act_func_set_id indices stay valid."""
    import concourse.bacc as bacc
    from concourse.hw_specs import get_activation_tables as _orig

    if getattr(bacc, "_act_tables_patched", False):
        return

    def _patched(arch):
        t = _orig(arch)
        return {k: (v if k == "natural_log_exp_and_others" else set())
                for k, v in t.items()}

    bacc.get_activation_tables = _patched
    bacc._act_tables_patched = True


def _build_nc(f_list):
    import concourse.bacc as bacc
    import concourse.mybir as mybir
    import concourse.tile as tile

    _patch_act_tables()

    key = tuple(f_list)
    if key in _NC_CACHE:
        return _NC_CACHE[key]
    S = P * sum(f_list)
    nc = bacc.Bacc("TRN2", target_bir_lowering=False, debug=False)
    x = nc.dram_tensor("x", (S, 16), mybir.dt.float16, kind="ExternalInput")
    out = nc.dram_tensor("out", (S, 16), mybir.dt.float16, kind="ExternalOutput")
    with nc.allow_low_precision(reason="fp16 kernel; 2e-2 rel-err budget"):
        with tile.TileContext(nc) as tc:
            _emit(tc, x.ap(), out.ap(), f_list)
    nc.compile()
    _NC_CACHE[key] = nc
    return nc


def kernel(x, _trace=False):
    from concourse.bass_utils import run_bass_kernel_spmd

    x = np.asarray(x)
    B = x.shape[0]
    assert x.shape == (B, 16) and B <= S_PAD
    xp = np.zeros((S_PAD, 16), dtype=np.float16)
    xp[:B] = x.astype(np.float16)
    shards = xp.reshape(N_CORES, S_CORE, 16)
    nc = _build_nc(F_LIST)
    in_maps = [{"x": np.ascontiguousarray(shards[i])} for i in range(N_CORES)]
    res = run_bass_kernel_spmd(nc, in_maps, core_ids=list(range(N_CORES)),
                               trace=_trace)
    o16 = np.concatenate([r["out"].reshape(S_CORE, 16) for r in res.results],
                         axis=0)[:B].astype(np.float32)
    full = np.empty((B, 4, 4, 2), dtype=np.float32)
    full[..., 0] = o16[:, _RE_IDX]
    full[..., 1] = o16[:, _IM_IDX] * _IM_SGN
    if _trace:
        return full, res
    return full


# revision 4
# speedup vs baseline: 1.5312x; 1.0091x over previous
"""Trainium2 Bass kernel for nn_CholeskyConstraintLayer — fp16 16-unique-value
design, software-pipelined emission.

Maps x:(B,16) f32 -> rho:(B,4,4,2) f32 where rho = L L^dagger / (trace + eps),
L lower-triangular complex 4x4 built from x (softplus diagonal, raw re/im
off-diagonals).

The device computes, per sample, the 16 unique values of the Hermitian rho in
the slot layout
  [r10,i10, r20,i20, r30,i30, re21,re31, im21,im31, re32,im32, d0,d1,d2,d3]
(chosen so every on-device writer gets a packed access pattern).  The host
casts fp16 -> f32 and mirrors the conjugate-symmetric full (B,4,4,2) tensor
(upper triangle = conj of lower, diagonal imag = 0).

All device IO is fp16 (input cast host-side): 32B in + 32B out per sample.
Work split: ACT does softplus (exp+ln), squares and the pair-swap copies; DVE
does the products, level-1 dot adds, diag-sum trees, trace/reciprocal and the
packed in-place normalize; POOL does the j0-column products, level-2/3 dot
adds and the s1 tree.

Emission is software-pipelined: the serial per-tile tail (trace -> rcp ->
normalize -> DMA out, all on DVE after two POOL partials) of tile t is
emitted between the ACT phase of tile t+1 and the products phase of tile t+1,
so no in-order engine queue ever stalls the next tile's work behind the tail
chain.

Per-sample math (y = x with softplus at 0,3,8,15; zsw = [y5,y4,y7,
y10,y9,y12,y11,y14]):
  re21,im21 from products y[4:7]*y[1:4] and zsw[0:3]*y[1:4] (im = p0-p1+p2)
  re31,im31 from y[9:12]*y[1:4] and zsw[3:6]*y[1:4]
  re32,im32 from y[9:14]*y[4:9] and zsw[3:8]*y[4:9] (im = p0-p1+p2-p3+p4)
  (i,0) column: (y1,y2)*y0, (y4,y5)*y0, (y9,y10)*y0
  d_i = sum of squares of row i;  trace = sum of all 16 squares.
"""

import numpy as np

P = 128
EPS = 1e-8
N_CORES = 8
BATCH = 1_000_000
F_LIST = [60, 220, 250, 250, 167, 30]  # sum = 977
S_CORE = P * sum(F_LIST)  # 125056
S_PAD = S_CORE * N_CORES  # 1000448

_NC_CACHE = {}

# Host-side conjugate mirror indices into the device slot layout.
_RE_IDX = np.array([[12, 0, 2, 4], [0, 13, 6, 7], [2, 6, 14, 10], [4, 7, 10, 15]])
_IM_IDX = np.array([[0, 1, 3, 5], [1, 0, 8, 9], [3, 8, 0, 11], [5, 9, 11, 0]])
_IM_SGN = np.array([[0., -1., -1., -1.], [1., 0., -1., -1.],
                    [1., 1., 0., -1.], [1., 1., 1., 0.]], dtype=np.float32)


def _emit(tc, x_ap, out_ap, f_list):
    import concourse.bass as bass
    import concourse.mybir as mybir
    from contextlib import ExitStack

    nc = tc.nc
    f16 = mybir.dt.float16
    f32 = mybir.dt.float32
    A = mybir.AluOpType
    ACT = mybir.ActivationFunctionType

    def ap3(view, offset, dims):
        """Custom AP rooted at a (P, F, e) view: dims = [stride, n] pairs in
        element units appended after the partition dim."""
        return bass.AP(tensor=view.tensor, offset=view.offset + offset,
                       ap=[list(view.ap[0])] + [list(d) for d in dims])

    starts = []
    s0 = 0
    for F in f_list:
        starts.append(s0)
        s0 += P * F

    with ExitStack() as ctx:
        tp = lambda name, bufs: ctx.enter_context(tc.tile_pool(name=name, bufs=bufs))
        ypool = tp("y", 3)
        sqpool = tp("sq", 3)
        zpool = tp("zsw", 3)
        prpool = tp("pr", 3)
        hpool = tp("h", 3)
        mpool = tp("misc", 3)
        opool = tp("out", 3)

        live = {}

        def emit_phase_a(ti):
            """DMA-in + all ACT work (softplus, squares, zsw)."""
            F = f_list[ti]
            s0 = starts[ti]
            y_t = ypool.tile([P, F * 16], f16, tag="y")
            xin = bass.AP(tensor=x_ap.tensor, offset=(s0 * 16),
                          ap=[[F * 16, P], [1, F * 16]])
            nc.sync.dma_start(y_t[:, :], xin)

            y = y_t[:, :].rearrange("p (f e) -> p f e", e=16)

            # softplus = Ln(Exp(x) + 1) on diag slots (sq as scratch)
            sq_t = sqpool.tile([P, F * 16], f16, tag="sq")
            sq = sq_t[:, :].rearrange("p (f e) -> p f e", e=16)
            for sl in (slice(3, 9, 5), slice(0, 16, 15)):
                nc.scalar.activation(sq[:, :, sl], y[:, :, sl], ACT.Exp)
                nc.scalar.activation(y[:, :, sl], sq[:, :, sl], ACT.Ln, bias=1.0)

            # squares: slots 1..15 into sq, sq0 straight into d0@12
            nc.scalar.activation(sq[:, :, 1:16], y[:, :, 1:16], ACT.Square)

            o_t = opool.tile([P, F * 16], f16, tag="o")
            o = o_t[:, :].rearrange("p (f e) -> p f e", e=16)
            nc.scalar.activation(o[:, :, 12:13], y[:, :, 0:1], ACT.Square)

            # zsw = [y5,y4,y7, y10,y9,y12,y11,y14] (pure copies; negations
            # fold into the subtract adds)
            z_t = zpool.tile([P, F * 8], f16, tag="z")
            z = z_t[:, :].rearrange("p (f e) -> p f e", e=8)
            nc.scalar.copy(ap3(z, 0, [[8, F], [1, 2]]),
                           ap3(y, 5, [[16, F], [-1, 2]]))          # y5,y4
            nc.scalar.copy(z[:, :, 2:3], y[:, :, 7:8])              # y7
            nc.scalar.copy(ap3(z, 3, [[8, F], [2, 2], [1, 2]]),
                           ap3(y, 10, [[16, F], [2, 2], [-1, 2]]))  # y10,y9,y12,y11
            nc.scalar.copy(z[:, :, 7:8], y[:, :, 14:15])            # y14

            live[ti] = dict(y=y, sq=sq, z=z, o=o, o_t=o_t, F=F, s0=s0)

        def emit_phase_b(ti):
            """Products, tree adds, diag sums (DVE + POOL)."""
            d = live[ti]
            y, sq, z, o, F = d["y"], d["sq"], d["z"], d["o"], d["F"]

            # POOL: j0 column, merged pairs: (r10,i10,r20,i20) then (r30,i30)
            y0b = y[:, :, 0:1].broadcast_to((P, F, 2))
            nc.gpsimd.tensor_tensor(o[:, :, 0:2], y[:, :, 1:3], y0b, op=A.mult)
            nc.gpsimd.tensor_tensor(o[:, :, 2:4], y[:, :, 4:6], y0b, op=A.mult)
            nc.gpsimd.tensor_tensor(o[:, :, 4:6], y[:, :, 9:11], y0b, op=A.mult)

            # DVE products into pr:
            # re21@0(3) im21@3(3) re31@6(3) im31@9(3) re32@12(5) im32@17(5)
            pr_t = prpool.tile([P, F * 22], f16, tag="pr")
            pr = pr_t[:, :].rearrange("p (f e) -> p f e", e=22)
            row1 = y[:, :, 1:4]
            nc.vector.tensor_tensor(pr[:, :, 0:3], y[:, :, 4:7], row1, op=A.mult)
            nc.vector.tensor_tensor(pr[:, :, 3:6], z[:, :, 0:3], row1, op=A.mult)
            nc.vector.tensor_tensor(pr[:, :, 6:9], y[:, :, 9:12], row1, op=A.mult)
            nc.vector.tensor_tensor(pr[:, :, 9:12], z[:, :, 3:6], row1, op=A.mult)
            row2 = y[:, :, 4:9]
            nc.vector.tensor_tensor(pr[:, :, 12:17], y[:, :, 9:14], row2, op=A.mult)
            nc.vector.tensor_tensor(pr[:, :, 17:22], z[:, :, 3:8], row2, op=A.mult)

            # tree adds.  h layout: k3h@0(4) k5h@4(4) k5l2@8(2)
            h_t = hpool.tile([P, F * 10], f16, tag="h")
            h = h_t[:, :].rearrange("p (f e) -> p f e", e=10)
            # k3 L1 (DVE): h[d] = p0 + p2 for dots (re21, im21, re31, im31)
            nc.vector.tensor_tensor(h[:, :, 0:4],
                                    ap3(pr, 0, [[22, F], [3, 4]]),
                                    ap3(pr, 2, [[22, F], [3, 4]]), op=A.add)
            # k5 L1 (DVE): (p0,p1)+(p2,p3) for re32, im32
            nc.vector.tensor_tensor(h[:, :, 4:8],
                                    ap3(pr, 12, [[22, F], [5, 2], [1, 2]]),
                                    ap3(pr, 14, [[22, F], [5, 2], [1, 2]]), op=A.add)
            # k3 L2 (POOL): (re21,re31)@6 = h{0,2}+pr{1,7}; (im21,im31)@8 = h{1,3}-pr{4,10}
            nc.gpsimd.tensor_tensor(o[:, :, 6:8],
                                    ap3(h, 0, [[10, F], [2, 2]]),
                                    ap3(pr, 1, [[22, F], [6, 2]]), op=A.add)
            nc.gpsimd.tensor_tensor(o[:, :, 8:10],
                                    ap3(h, 1, [[10, F], [2, 2]]),
                                    ap3(pr, 4, [[22, F], [6, 2]]), op=A.subtract)
            # k5 L2 (POOL): h8 = h4+h5 (re), h9 = h6-h7 (im)
            nc.gpsimd.tensor_tensor(h[:, :, 8:9], h[:, :, 4:5], h[:, :, 5:6], op=A.add)
            nc.gpsimd.tensor_tensor(h[:, :, 9:10], h[:, :, 6:7], h[:, :, 7:8], op=A.subtract)
            # k5 L3 (POOL): (re32,im32)@10 = h[8:10] + pr{16,21}
            nc.gpsimd.tensor_tensor(o[:, :, 10:12], h[:, :, 8:10],
                                    ap3(pr, 16, [[22, F], [5, 2]]), op=A.add)

            # diag sums: s1 tree on POOL, s2/s3 trees on DVE
            # m layout: u@0, g2@(1,2), v@3, g3@(4,5,6), w@7, tp@(8,9)
            m_t = mpool.tile([P, F * 10], f16, tag="m")
            mv = m_t[:, :].rearrange("p (f e) -> p f e", e=10)
            u = mv[:, :, 0:1]
            nc.gpsimd.tensor_tensor(u, sq[:, :, 1:2], sq[:, :, 2:3], op=A.add)
            nc.gpsimd.tensor_tensor(o[:, :, 13:14], u, sq[:, :, 3:4], op=A.add)
            # s2 = sq4+..+sq8, s3 = sq9+..+sq15
            nc.vector.tensor_tensor(mv[:, :, 1:3], sq[:, :, 4:6], sq[:, :, 6:8], op=A.add)
            nc.vector.tensor_tensor(mv[:, :, 4:7], sq[:, :, 9:12], sq[:, :, 12:15], op=A.add)
            # v = g2a+g2b ; w = g3a+g3b   (one op via stride-3 pairs)
            nc.vector.tensor_tensor(ap3(mv, 3, [[10, F], [4, 2]]),
                                    ap3(mv, 1, [[10, F], [3, 2]]),
                                    ap3(mv, 2, [[10, F], [3, 2]]), op=A.add)
            # sq15 += g3c  (so the packed L3 reads (sq8, sq15))
            nc.vector.tensor_tensor(sq[:, :, 15:16], mv[:, :, 6:7], sq[:, :, 15:16], op=A.add)
            # (d2,d3)@14 = (v, w) + (sq8, sq15)
            nc.vector.tensor_tensor(o[:, :, 14:16],
                                    ap3(mv, 3, [[10, F], [4, 2]]),
                                    ap3(sq, 8, [[16, F], [7, 2]]), op=A.add)
            d["mv"] = mv

        def emit_tail(ti):
            """Trace -> reciprocal -> normalize -> DMA out, all on DVE (single
            stream, no cross-engine queue-head stalls)."""
            d = live.pop(ti)
            o, o_t, F, s0, mv = d["o"], d["o_t"], d["F"], d["s0"], d["mv"]
            # tp = (d0+d2, d1+d3) over the packed diag block
            tp_ = mv[:, :, 8:10]
            nc.vector.tensor_tensor(tp_, o[:, :, 12:14], o[:, :, 14:16], op=A.add)
            mf_t = mpool.tile([P, F * 2], f32, tag="mf")
            mf = mf_t[:, :].rearrange("p (f e) -> p f e", e=2)
            trE, rcp = mf[:, :, 0:1], mf[:, :, 1:2]
            nc.vector.scalar_tensor_tensor(trE, mv[:, :, 8:9], float(EPS),
                                           mv[:, :, 9:10], op0=A.add, op1=A.add)
            nc.vector.reciprocal_approx_fast(rcp, trE)

            # rp: fp16 duplicated pair of rcp for the packed normalize bcast
            rp_t = mpool.tile([P, F * 2], f16, tag="rp")
            rp = rp_t[:, :].rearrange("p (f e) -> p f e", e=2)
            nc.vector.tensor_copy(rp[:, :, 0:2], rcp.broadcast_to((P, F, 2)))

            # normalize all 16 slots in place
            rpb = bass.AP(tensor=rp.tensor, offset=rp.offset,
                          ap=[list(rp.ap[0]), [2, F], [0, 8], [1, 2]])
            ov16 = bass.AP(tensor=o.tensor, offset=o.offset,
                           ap=[list(o.ap[0]), [16, F], [2, 8], [1, 2]])
            nc.vector.tensor_tensor(ov16, ov16, rpb, op=A.mult)

            # DMA out, triggered from the ACT queue (keeps SP free-running for
            # the in-DMAs; ACT reaches this after the next tile's phase A, by
            # which time the normalize is done)
            odst = bass.AP(tensor=out_ap.tensor, offset=(s0 * 16),
                           ap=[[F * 16, P], [1, F * 16]])
            nc.scalar.dma_start(odst, o_t[:, :])

        T = len(f_list)
        for t in range(T + 1):
            if t < T:
                emit_phase_a(t)
            if t >= 1:
                emit_tail(t - 1)
            if t < T:
                emit_phase_b(t)


def _patch_act_tables():
    """Force every ACT function onto the one table set that contains all of
    Exp/Ln/Square/Copy, so the table-load pass emits a single load instead of
    ping-ponging exp_and_others <-> natural_log every tile (~2.7us per load).
    Keys keep their order so act_func_set_id indices stay valid."""
    import concourse.bacc as bacc
    from concourse.hw_specs import get_activation_tables as _orig

    if getattr(bacc, "_act_tables_patched", False):
        return

    def _patched(arch):
        t = _orig(arch)
        return {k: (v if k == "natural_log_exp_and_others" else set())
                for k, v in t.items()}

    bacc.get_activation_tables = _patched
    bacc._act_tables_patched = True


def _build_nc(f_list):
    import concourse.bacc as bacc
    import concourse.mybir as mybir
    import concourse.tile as tile

    _patch_act_tables()

    key = tuple(f_list)
    if key in _NC_CACHE:
        return _NC_CACHE[key]
    S = P * sum(f_list)
    nc = bacc.Bacc("TRN2", target_bir_lowering=False, debug=False)
    x = nc.dram_tensor("x", (S, 16), mybir.dt.float16, kind="ExternalInput")
    out = nc.dram_tensor("out", (S, 16), mybir.dt.float16, kind="ExternalOutput")
    with nc.allow_low_precision(reason="fp16 kernel; 2e-2 rel-err budget"):
        with tile.TileContext(nc) as tc:
            _emit(tc, x.ap(), out.ap(), f_list)
    nc.compile()
    _NC_CACHE[key] = nc
    return nc


def kernel(x, _trace=False):
    from concourse.bass_utils import run_bass_kernel_spmd

    x = np.asarray(x)
    B = x.shape[0]
    assert x.shape == (B, 16) and B <= S_PAD
    xp = np.zeros((S_PAD, 16), dtype=np.float16)
    xp[:B] = x.astype(np.float16)
    shards = xp.reshape(N_CORES, S_CORE, 16)
    nc = _build_nc(F_LIST)
    in_maps = [{"x": np.ascontiguousarray(shards[i])} for i in range(N_CORES)]
    res = run_bass_kernel_spmd(nc, in_maps, core_ids=list(range(N_CORES)),
                               trace=_trace)
    o16 = np.concatenate([r["out"].reshape(S_CORE, 16) for r in res.results],
                         axis=0)[:B].astype(np.float32)
    full = np.empty((B, 4, 4, 2), dtype=np.float32)
    full[..., 0] = o16[:, _RE_IDX]
    full[..., 1] = o16[:, _IM_IDX] * _IM_SGN
    if _trace:
        return full, res
    return full
